# revision 26
# baseline (speedup 1.0000x reference)
"""Trainium2 Bass kernel for nn_ExampleModel_9234179686517 (dense_mlp).

Model: bilinear grid-sample of a (4, 512, 512) featuremap at 4M points,
concat with xyz, then a 7->16->16->16->16->3 ReLU MLP.

Strategy (pure data parallel over 8 NeuronCores):
 - Host precomputes a bf16 "window table" Qb[y*64+xb] = pair-rows y,y+1
   (border-clamped) for the 16 x-positions [8*xb, 8*xb+16), all 4
   channels: 2*16*4 bf16 = 256B per row, 32768 rows (15-bit int16 idx).
 - Each core gathers one 256B row per point with the GPSIMD dma_gather
   ucode (1024 idxs per call, the ucode's hard limit). Each call runs on
   the Q7 core pair selected by queue_num, so consecutive calls are
   striped across all 4 SWDGE queues -- the Pool engine pipelines them on
   different core pairs, cutting the gather wall ~4x. (The alternative
   primitives ap_gather / indirect_copy / indirect_dma_start are broken
   on this deployment.)
 - Gather indices are computed once in the main 128-partition pipeline
   (reusing the coordinate floors) and reshuffled to the gather's
   16-partition wrapped layout via a DRAM bounce, a small DVE free-dim
   transpose, and a replicating DRAM read -- this replaces the baseline's
   separate 16-partition coordinate recompute pipeline.
 - The 3-bit x sub-position is resolved with a 3-level bf16 shift tree
   in merged 4-dim ops (DVE per-instruction overhead dominates, so one
   op per level, not per pair-row): level-1 copy + all predicated
   overwrites on VectorE, level-2/3 copies on ScalarE (a tiny DVE guard
   read pins the cross-engine copy<pred RMW order), then bilinear lerp
   in bf16 on VectorE.  MLP bias+ReLU runs on ScalarE as single fused
   [P,1024] activations, and each MLP group is interleaved directly
   after its select chunk.
 - TensorE transposes point-major -> feature-major and runs the MLP as
   block-diagonal (8 networks wide) bf16 matmuls with fp32 PSUM
   accumulation; ReLU+bias on ScalarE/VectorE; results un-transposed on
   TensorE and DMAed back as f32.
 - Slot tiles are 7x512 + 384 per lane (3968), cutting padded points to
   1.6% (vs 4.9% in the baseline's uniform 4096).
"""

import sys

for _p in ("/opt/trn_rl_repo", "/root/.axon_site/_ro/trn_rl_repo"):
    if _p not in sys.path:
        sys.path.insert(0, _p)

import numpy as np
import ml_dtypes

BF16 = ml_dtypes.bfloat16

N_TOTAL = 4_000_000
N_CORES = 8
C, H, W = 4, 512, 512
HID = 16

P = 128          # partitions
GS = 64          # slots per lane per MLP group (8192 points)
FPAD = 8         # padded feature count (7 real + 1 zero)
TCH = 64         # slots per gather/select chunk (8192 points)
GNI = 1024       # idxs per dma_gather call
SCRATCH = 16384  # SWDGE ring bytes/partition (1024 descriptors)

S_TILES = [512] * 7 + [384]          # per-iteration slot tiles
M_SLOTS = sum(S_TILES)               # 3968 slots per lane
N_CORE = N_TOTAL // N_CORES          # 500_000
N_PAD = P * M_SLOTS                  # 507_904 padded points per core
SMAX = max(S_TILES)

NROWS = 512 * 64                     # window-table rows (= 32768)


def _build_host_constants(featuremap, Ws, bs):
    """Window table + block-diagonal bf16 weights."""
    fmT = np.ascontiguousarray(featuremap.transpose(1, 2, 0)).astype(np.float32)
    ys = np.arange(H)
    y2 = np.stack([ys, np.minimum(ys + 1, H - 1)], 1)            # [512, 2]
    xs = (np.arange(64)[:, None] * 8 + np.arange(16)[None, :])   # [64, 16]
    xs = np.minimum(xs, W - 1)
    qtab = fmT[y2[:, None, :, None], xs[None, :, None, :], :]    # [512, 64, 2, 16, 4]
    qtab = qtab.reshape(NROWS, 128).astype(BF16)

    W1, W2, W3, W4, W5 = Ws
    b1, b2, b3, b4, b5 = bs

    W1a = np.zeros((FPAD, HID), np.float32)
    W1a[:7] = W1

    def blockdiag(Wm, nb):
        fi, fo = Wm.shape
        out = np.zeros((fi * nb, fo * nb), np.float32)
        for b in range(nb):
            out[b * fi:(b + 1) * fi, b * fo:(b + 1) * fo] = Wm
        return out

    w1blk = blockdiag(W1a, 8)                      # [64, 128]
    w1stack = np.concatenate([w1blk, w1blk], 0)    # [128, 128]

    return {
        "qtab": qtab,
        "w1stack": w1stack.astype(BF16),
        "w2blk": blockdiag(W2, 8).astype(BF16),
        "w3blk": blockdiag(W3, 8).astype(BF16),
        "w4blk": blockdiag(W4, 8).astype(BF16),
        "w5blk": blockdiag(W5, 8).astype(BF16),
        "b1blk": np.tile(b1, 8).reshape(P, 1).astype(np.float32),
        "b2blk": np.tile(b2, 8).reshape(P, 1).astype(np.float32),
        "b3blk": np.tile(b3, 8).reshape(P, 1).astype(np.float32),
        "b4blk": np.tile(b4, 8).reshape(P, 1).astype(np.float32),
        "b5blk": np.tile(b5, 8).reshape(24, 1).astype(np.float32),
        "id128": np.eye(P, dtype=np.float32).astype(BF16),
        "id24": np.eye(24, dtype=np.float32),
    }


def build_program(s_tiles=None, gather=True, mlp=True):
    """Build the per-core Bass program (same program for all 8 cores)."""
    import concourse.bass as bass
    import concourse.tile as tile
    from concourse import bacc, mybir

    f32 = mybir.dt.float32
    bf16 = mybir.dt.bfloat16
    i16 = mybir.dt.int16
    u8d = mybir.dt.uint8
    AF = mybir.ActivationFunctionType
    OP = mybir.AluOpType

    if s_tiles is None:
        s_tiles = S_TILES
    n_slots = sum(s_tiles)
    n_pad = P * n_slots
    smax = max(s_tiles)
    for st in s_tiles:
        assert st % GS == 0 and st % TCH == 0

    nc = bacc.Bacc("TRN2", target_bir_lowering=False, debug=False,
                   enable_asserts=False, num_devices=N_CORES,
                   dynamic_dma_scratch_size=SCRATCH, num_swdge_queues=4)

    xin = nc.dram_tensor("x", [n_pad, 3], f32, kind="ExternalInput").ap()
    qtab = nc.dram_tensor("qtab", [NROWS, 128], bf16, kind="ExternalInput").ap()
    w1stack = nc.dram_tensor("w1stack", [P, P], bf16, kind="ExternalInput").ap()
    w2 = nc.dram_tensor("w2blk", [P, P], bf16, kind="ExternalInput").ap()
    w3 = nc.dram_tensor("w3blk", [P, P], bf16, kind="ExternalInput").ap()
    w4 = nc.dram_tensor("w4blk", [P, P], bf16, kind="ExternalInput").ap()
    w5 = nc.dram_tensor("w5blk", [P, 24], bf16, kind="ExternalInput").ap()
    b1 = nc.dram_tensor("b1blk", [P, 1], f32, kind="ExternalInput").ap()
    b2i = nc.dram_tensor("b2blk", [P, 1], f32, kind="ExternalInput").ap()
    b3i = nc.dram_tensor("b3blk", [P, 1], f32, kind="ExternalInput").ap()
    b4i = nc.dram_tensor("b4blk", [P, 1], f32, kind="ExternalInput").ap()
    b5i = nc.dram_tensor("b5blk", [24, 1], f32, kind="ExternalInput").ap()
    id128 = nc.dram_tensor("id128", [P, P], bf16, kind="ExternalInput").ap()
    id24 = nc.dram_tensor("id24", [24, 24], f32, kind="ExternalInput").ap()
    yout = nc.dram_tensor("y", [n_pad, 3], bf16, kind="ExternalOutput").ap()

    # lane p owns rows [p*n_slots, (p+1)*n_slots)  (contiguous HBM runs)
    xv = xin.rearrange("(p s) c -> p s c", p=P)
    yv = yout.rearrange("(p s) c -> p s c", p=P)

    BIGF = float(2 ** 23)

    from contextlib import ExitStack

    with tile.TileContext(nc) as tc, ExitStack() as ctx:
        ep = ctx.enter_context
        consts = ep(tc.tile_pool(name="consts", bufs=1))
        xio = ep(tc.tile_pool(name="xio", bufs=2))
        ctmp = ep(tc.tile_pool(name="ctmp", bufs=2))      # iter-local coords
        cper = ep(tc.tile_pool(name="cper", bufs=2))      # persists thru chunks
        jidxp = ep(tc.tile_pool(name="jidx", bufs=2))
        gatp = ep(tc.tile_pool(name="gat", bufs=3))
        shiftp = ep(tc.tile_pool(name="shift", bufs=1))
        stagep = ep(tc.tile_pool(name="stage", bufs=2))
        tsbp = ep(tc.tile_pool(name="tsb", bufs=2))
        actsp = ep(tc.tile_pool(name="acts", bufs=2))
        s5p = ep(tc.tile_pool(name="s5", bufs=1))
        ostagep = ep(tc.tile_pool(name="ostage", bufs=2))
        ptr = ep(tc.tile_pool(name="ptr", bufs=2, space="PSUM"))
        pmm = ep(tc.tile_pool(name="pmm", bufs=2, space="PSUM"))
        p5 = ep(tc.tile_pool(name="p5", bufs=1, space="PSUM"))
        dramp = ep(tc.tile_pool(name="dram", bufs=2, space="DRAM"))

        # ---- constants into SBUF
        w1_sb = consts.tile([P, P], bf16, tag="w1")
        w2_sb = consts.tile([P, P], bf16, tag="w2")
        w3_sb = consts.tile([P, P], bf16, tag="w3")
        w4_sb = consts.tile([P, P], bf16, tag="w4")
        w5_sb = consts.tile([P, 24], bf16, tag="w5")
        b1_sb = consts.tile([P, 1], f32, tag="b1")
        b2_sb = consts.tile([P, 1], f32, tag="b2")
        b3_sb = consts.tile([P, 1], f32, tag="b3")
        b4_sb = consts.tile([P, 1], f32, tag="b4")
        b5_sb = consts.tile([24, 1], f32, tag="b5")
        id128_sb = consts.tile([P, P], bf16, tag="id128")
        id24_sb = consts.tile([24, 24], f32, tag="id24")
        cm05 = consts.tile([P, 1], f32, tag="cm05")
        nc.vector.memset(cm05[:], -0.5)
        cm16 = consts.tile([P, 1], f32, tag="cm16")
        nc.vector.memset(cm16[:], -0.0625)
        for sb, src in (
            (w1_sb, w1stack), (w2_sb, w2), (w3_sb, w3), (w4_sb, w4),
            (w5_sb, w5), (b1_sb, b1), (b2_sb, b2i), (b3_sb, b3i),
            (b4_sb, b4i), (b5_sb, b5i), (id128_sb, id128), (id24_sb, id24),
        ):
            nc.sync.dma_start(out=sb[:], in_=src)

        def floor_exact(fsrc, tagp):
            """floor(fsrc); exact for f in [0, 2^22)."""
            b_ = ctmp.tile([P, smax], f32, tag=f"fb{tagp}", name=f"fb{tagp}")[:, :fsrc.shape[1]]
            nc.vector.tensor_scalar(out=b_, in0=fsrc, scalar1=BIGF,
                                    scalar2=BIGF, op0=OP.add, op1=OP.subtract)
            cgt = ctmp.tile([P, smax], f32, tag="fc", name="fc")[:, :fsrc.shape[1]]
            nc.vector.tensor_tensor(out=cgt, in0=b_, in1=fsrc, op=OP.is_gt)
            nc.vector.tensor_tensor(out=b_, in0=b_, in1=cgt, op=OP.subtract)
            return b_

        sl0 = 0
        gq = [0]
        for it, s_tile in enumerate(s_tiles):
            chunks = s_tile // TCH
            groups = s_tile // GS
            gcalls = P * TCH // GNI          # gather calls per chunk (4)
            gw = GNI // 16                   # idx cols per gather call (128)

            # ======== coordinate pipeline (point-major) ========
            xt = xio.tile([P, smax, 3], f32, tag="xt", name="xt")[:, :s_tile]
            nc.sync.dma_start(out=xt, in_=xv[:, sl0:sl0 + s_tile, :])

            fx = ctmp.tile([P, smax], f32, tag="fx", name="fx")[:, :s_tile]
            nc.scalar.activation(out=fx, in_=xt[:, :, 0], func=AF.Relu,
                                 bias=cm05[:], scale=float(W))
            fy = ctmp.tile([P, smax], f32, tag="fy", name="fy")[:, :s_tile]
            nc.scalar.activation(out=fy, in_=xt[:, :, 1], func=AF.Relu,
                                 bias=cm05[:], scale=float(H))
            u8 = ctmp.tile([P, smax], f32, tag="u8", name="u8")[:, :s_tile]
            nc.scalar.activation(out=u8, in_=xt[:, :, 0], func=AF.Relu,
                                 bias=cm16[:], scale=float(W) / 8.0)

            xbf = floor_exact(u8, "x")
            iyf = floor_exact(fy, "y")

            # gather idx = 64*iyf + xbf -> int16, reshuffled to the gather's
            # 16-partition wrapped layout: idx[16r+p, sl*8+Lhi] =
            # idxpm[16*Lhi+p, sl].  DMA descriptors are 3-dim with contiguous
            # inner, so do it as: DRAM bounce to [16, Lhi, sl], a DVE
            # free-dim transpose to [16, sl, Lhi], then a DRAM hop that also
            # replicates x8 for the Q7 core groups.
            idxf = ctmp.tile([P, smax], f32, tag="idxf", name="idxf")[:, :s_tile]
            nc.vector.scalar_tensor_tensor(out=idxf, in0=iyf, scalar=64.0,
                                           in1=xbf, op0=OP.mult, op1=OP.add)
            jidx16 = ctmp.tile([P, smax], i16, tag="jidx16", name="jidx16")[:, :s_tile]
            nc.vector.tensor_copy(out=jidx16, in_=idxf)
            jd = dramp.tile([P, smax], i16, tag="jd")
            nc.sync.dma_start(out=jd[:, :s_tile], in_=jidx16)
            jda = jd[:]
            jd2 = dramp.tile([16, smax * 8], i16, tag="jd2")
            hmax = smax // 2
            for hf in range(2):
                hs = s_tile // 2
                h0 = hf * hs
                j2 = ctmp.tile([16, 8, hmax], i16, tag="j2", name="j2")[:, :, :hs]
                nc.sync.dma_start(out=j2, in_=bass.AP(
                    tensor=jda.tensor, offset=jda.offset + h0,
                    ap=[[smax, 16], [16 * smax, 8], [1, hs]]))
                j3 = ctmp.tile([16, hmax * 8], i16, tag="j3", name="j3")[:, :hs * 8]
                nc.vector.tensor_copy(
                    out=j3.rearrange("p (s l) -> p s l", l=8),
                    in_=j2.rearrange("p l s -> p s l"))
                nc.sync.dma_start(out=jd2[:, h0 * 8:(h0 + hs) * 8], in_=j3)
            jidx = jidxp.tile([P, smax * 8], i16, tag="jidx")
            jd2a = jd2[:]
            nc.sync.dma_start(out=jidx[:, :s_tile * 8], in_=bass.AP(
                tensor=jd2a.tensor, offset=jd2a.offset,
                ap=[[0, 8], [smax * 8, 16], [1, s_tile * 8]]))

            # q bits + lerp weights
            u = ctmp.tile([P, smax], f32, tag="u", name="u")[:, :s_tile]
            nc.vector.scalar_tensor_tensor(out=u, in0=xbf, scalar=-8.0,
                                           in1=fx, op0=OP.mult, op1=OP.add)
            b2f = cper.tile([P, smax], f32, tag="b2f", name="b2f")[:, :s_tile]
            nc.vector.tensor_scalar(out=b2f, in0=u, scalar1=4.0,
                                    scalar2=None, op0=OP.is_ge)
            u2 = ctmp.tile([P, smax], f32, tag="u2", name="u2")[:, :s_tile]
            nc.vector.scalar_tensor_tensor(out=u2, in0=b2f, scalar=-4.0,
                                           in1=u, op0=OP.mult, op1=OP.add)
            b1f = cper.tile([P, smax], f32, tag="b1f", name="b1f")[:, :s_tile]
            nc.vector.tensor_scalar(out=b1f, in0=u2, scalar1=2.0,
                                    scalar2=None, op0=OP.is_ge)
            u3 = ctmp.tile([P, smax], f32, tag="u3", name="u3")[:, :s_tile]
            nc.vector.scalar_tensor_tensor(out=u3, in0=b1f, scalar=-2.0,
                                           in1=u2, op0=OP.mult, op1=OP.add)
            b0f = cper.tile([P, smax], f32, tag="b0f", name="b0f")[:, :s_tile]
            nc.vector.tensor_scalar(out=b0f, in0=u3, scalar1=1.0,
                                    scalar2=None, op0=OP.is_ge)
            wx = cper.tile([P, smax], bf16, tag="wx", name="wx")[:, :s_tile]
            nc.vector.tensor_tensor(out=wx, in0=u3, in1=b0f, op=OP.subtract)
            mb2 = cper.tile([P, smax], u8d, tag="mb2", name="mb2")[:, :s_tile]
            nc.vector.tensor_copy(out=mb2, in_=b2f)
            mb1 = cper.tile([P, smax], u8d, tag="mb1", name="mb1")[:, :s_tile]
            nc.vector.tensor_copy(out=mb1, in_=b1f)
            mb0 = cper.tile([P, smax], u8d, tag="mb0", name="mb0")[:, :s_tile]
            nc.vector.tensor_copy(out=mb0, in_=b0f)
            wy = cper.tile([P, smax], bf16, tag="wy", name="wy")[:, :s_tile]
            nc.vector.tensor_tensor(out=wy, in0=fy, in1=iyf, op=OP.subtract)

            # ======== staging ========
            stg = stagep.tile([P, smax, FPAD], bf16, tag="stg", name="stg")[:, :s_tile]
            nc.vector.memset(stg[:, :, 7], 0.0)
            nc.scalar.activation(out=stg[:, :, 0:3], in_=xt,
                                 func=AF.Copy, bias=0.0, scale=1.0)

            # ======== gather + select + lerp per chunk ========
            for ch in range(chunks):
                cs = ch * TCH
                G = gatp.tile([P, TCH, 128], bf16, tag="G")
                if gather:
                    gsl = GNI // P       # slots per gather call (16)
                    for k in range(gcalls):
                        c0 = ch * TCH * 8 + k * gw
                        nc.gpsimd.dma_gather(
                            out_ap=G[:, k * gsl:(k + 1) * gsl, :],
                            in_ap=qtab,
                            idxs_ap=jidx[:, c0:c0 + gw],
                            num_idxs=GNI, num_idxs_reg=GNI, elem_size=128,
                            queue_num=gq[0] % 4)
                        gq[0] += 1
                else:
                    nc.vector.memset(G[:], 0.25)

                # fp32-pair view of the window: [P, TCH, 2 rows, 32 pairs]
                Gf = G[:].bitcast(f32).rearrange("p t (r e) -> p t r e", r=2)
                m2v = mb2[:, cs:cs + TCH, None]
                m1v = mb1[:, cs:cs + TCH, None]
                m0v = mb0[:, cs:cs + TCH, None]

                W1t = shiftp.tile([P, TCH, 2, 10], f32, tag="W1")
                W2t = shiftp.tile([P, TCH, 2, 6], f32, tag="W2")
                W3t = shiftp.tile([P, TCH, 2, 4], f32, tag="W3")
                for r in range(2):
                    nc.scalar.activation(out=W1t[:, :, r, :],
                                         in_=Gf[:, :, r, 0:10],
                                         func=AF.Copy, bias=0.0, scale=1.0)
                    nc.vector.copy_predicated(
                        out=W1t[:, :, r, :],
                        mask=m2v.to_broadcast([P, TCH, 10]),
                        data=Gf[:, :, r, 16:26])
                for r in range(2):
                    nc.scalar.activation(out=W2t[:, :, r, :],
                                         in_=W1t[:, :, r, 0:6],
                                         func=AF.Copy, bias=0.0, scale=1.0)
                    nc.vector.copy_predicated(
                        out=W2t[:, :, r, :],
                        mask=m1v.to_broadcast([P, TCH, 6]),
                        data=W1t[:, :, r, 4:10])
                for r in range(2):
                    nc.scalar.activation(out=W3t[:, :, r, :],
                                         in_=W2t[:, :, r, 0:4],
                                         func=AF.Copy, bias=0.0, scale=1.0)
                    nc.vector.copy_predicated(
                        out=W3t[:, :, r, :],
                        mask=m0v.to_broadcast([P, TCH, 4]),
                        data=W2t[:, :, r, 2:6])

                # lerp x then y -> staging features
                W3b = W3t[:].bitcast(bf16)        # [P, TCH, 2, 8]
                wxv = wx[:, cs:cs + TCH, None, None].to_broadcast([P, TCH, 2, 4])
                wyv = wy[:, cs:cs + TCH, None].to_broadcast([P, TCH, 4])
                d = shiftp.tile([P, TCH, 2, 4], bf16, tag="d")
                nc.vector.tensor_tensor(out=d[:], in0=W3b[:, :, :, 4:8],
                                        in1=W3b[:, :, :, 0:4], op=OP.subtract)
                nc.vector.tensor_tensor(out=d[:], in0=d[:], in1=wxv, op=OP.mult)
                nc.vector.tensor_tensor(out=d[:], in0=W3b[:, :, :, 0:4],
                                        in1=d[:], op=OP.add)
                e = d[:, :, 1, :]
                nc.vector.tensor_tensor(out=e, in0=d[:, :, 1, :],
                                        in1=d[:, :, 0, :], op=OP.subtract)
                nc.vector.tensor_tensor(out=e, in0=e, in1=wyv, op=OP.mult)
                nc.vector.tensor_tensor(out=stg[:, cs:cs + TCH, 3:7],
                                        in0=d[:, :, 0, :], in1=e, op=OP.add)

            ost = ostagep.tile([P, smax, 3], bf16, tag="ost", name="ost")[:, :s_tile]
            stg_flat = stg.rearrange("p s f -> p (s f)")

            if not mlp:
                nc.vector.tensor_copy(out=ost, in_=stg[:, :, 3:6])
                nc.sync.dma_start(out=yv[:, sl0:sl0 + s_tile, :], in_=ost)
                sl0 += s_tile
                continue

            # ======== MLP groups (GS slots = 8192 points each) ========
            for g in range(groups):
                t_ps = ptr.tile([P, 4, P], bf16, tag="tp")
                for c4 in range(4):
                    base = (g * GS + c4 * 16) * FPAD
                    nc.tensor.transpose(out=t_ps[:, c4, :],
                                        in_=stg_flat[:, base:base + P],
                                        identity=id128_sb[:])
                t_sb = tsbp.tile([P, 4, P], bf16, tag="tsb")
                nc.scalar.activation(out=t_sb[:], in_=t_ps[:],
                                     func=AF.Copy, bias=0.0, scale=1.0)

                ps = pmm.tile([P, 1024], f32, tag="ps")
                for c4 in range(4):
                    nc.tensor.matmul(out=ps[:, c4 * P:(c4 + 1) * P],
                                     lhsT=w1_sb[0:64, :], rhs=t_sb[0:64, c4, :],
                                     start=True, stop=True)
                    nc.tensor.matmul(out=ps[:, 512 + c4 * P:512 + (c4 + 1) * P],
                                     lhsT=w1_sb[64:128, :], rhs=t_sb[64:128, c4, :],
                                     start=True, stop=True)
                h = actsp.tile([P, 1024], bf16, tag="h")
                nc.scalar.activation(out=h[:, 0:512], in_=ps[:, 0:512],
                                     func=AF.Relu, bias=b1_sb[:], scale=1.0)
                nc.vector.tensor_scalar(out=h[:, 512:1024], in0=ps[:, 512:1024],
                                        scalar1=b1_sb[:], scalar2=0.0,
                                        op0=OP.add, op1=OP.max)

                for w_sb, bias_sb in ((w2_sb, b2_sb), (w3_sb, b3_sb), (w4_sb, b4_sb)):
                    ps = pmm.tile([P, 1024], f32, tag="ps")
                    nc.tensor.matmul(out=ps[:, 0:512], lhsT=w_sb[:], rhs=h[:, 0:512],
                                     start=True, stop=True)
                    nc.tensor.matmul(out=ps[:, 512:1024], lhsT=w_sb[:],
                                     rhs=h[:, 512:1024], start=True, stop=True)
                    h = actsp.tile([P, 1024], bf16, tag="h")
                    nc.scalar.activation(out=h[:, 0:512], in_=ps[:, 0:512],
                                         func=AF.Relu, bias=bias_sb[:], scale=1.0)
                    nc.vector.tensor_scalar(out=h[:, 512:1024], in0=ps[:, 512:1024],
                                            scalar1=bias_sb[:], scalar2=0.0,
                                            op0=OP.add, op1=OP.max)

                ps5 = p5.tile([24, 1024], f32, tag="ps5")
                nc.tensor.matmul(out=ps5[:, 0:512], lhsT=w5_sb[:], rhs=h[:, 0:512],
                                 start=True, stop=True)
                nc.tensor.matmul(out=ps5[:, 512:1024], lhsT=w5_sb[:],
                                 rhs=h[:, 512:1024], start=True, stop=True)
                s5 = s5p.tile([24, 1024], f32, tag="s5")
                nc.scalar.activation(out=s5[:], in_=ps5[:], func=AF.Identity,
                                     bias=b5_sb[:], scale=1.0)

                u_ps = ptr.tile([P, 8, 24], f32, tag="tp")
                for ui in range(2):
                    for c4 in range(4):
                        nc.tensor.transpose(
                            out=u_ps[:, c4 * 2 + ui, :],
                            in_=s5[:, ui * 512 + c4 * P: ui * 512 + (c4 + 1) * P],
                            identity=id24_sb[:])
                uv = u_ps.rearrange("p k (b c) -> p k b c", c=3)
                ostg = ost[:, g * GS:(g + 1) * GS, :].rearrange(
                    "p (c u b) d -> p c u b d", c=4, u=2)
                for ui in range(2):
                    nc.vector.tensor_copy(out=ostg[:, :, ui, :, :],
                                          in_=uv[:, ui::2, :, :])

            nc.sync.dma_start(out=yv[:, sl0:sl0 + s_tile, :], in_=ost)
            sl0 += s_tile

    nc.compile()
    return nc


_PROGRAM_CACHE = {}


def _get_program(*args):
    if "prog" not in _PROGRAM_CACHE:
        _PROGRAM_CACHE["prog"] = build_program()
    return _PROGRAM_CACHE["prog"]


def make_in_maps(x_full, consts, n_cores=N_CORES):
    per = x_full.shape[0] // n_cores
    in_maps = []
    for c in range(n_cores):
        xpad = np.zeros((N_PAD, 3), np.float32)
        xpad[:per] = x_full[c * per:(c + 1) * per]
        in_maps.append({"x": xpad, **{k: np.ascontiguousarray(v)
                                      for k, v in consts.items()}})
    return in_maps


def kernel(**inputs):
    from concourse import bass_utils
    from concourse.bass_interp import get_hw_module

    x = np.asarray(inputs["x"], dtype=np.float32)
    fm = np.asarray(inputs["featuremap"], dtype=np.float32)
    Ws = [np.asarray(inputs[f"W{i}"], dtype=np.float32) for i in range(1, 6)]
    bs = [np.asarray(inputs[f"b{i}"], dtype=np.float32) for i in range(1, 6)]

    consts = _build_host_constants(fm, Ws, bs)
    n = x.shape[0]
    assert n == N_TOTAL, n
    per = n // N_CORES

    nc = _get_program()
    old_m = nc.m
    nc.m = get_hw_module(nc.m)
    try:
        in_maps = make_in_maps(x, consts)
        res = bass_utils.run_bass_kernel_spmd(nc, in_maps,
                                              core_ids=list(range(N_CORES)))
    finally:
        nc.m = old_m
    outs = [r["y"][:per].astype(np.float32) for r in res.results]
    return np.concatenate(outs, axis=0)


if __name__ == "__main__":
    build_program([128, 128])
    print("small program built OK")
